# revision 17
# baseline (speedup 1.0000x reference)
"""Trainium2 Bass kernel for the DifferentiableQuantumCircuit problem.

Math: output = |U x / ||x|| |^2 with U = kron of 12 single-qubit U3 gates
applied twice (2 layers). Gates on different qubits commute, so the two
layers fuse into ONE kron-product unitary with per-qubit gates
G_q = U3_layer2(q) @ U3_layer1(q).

Host side: x is pre-normalized (x / ||x||, numpy) and the gate constants
are built in float64 then cast to bf16 (same precedent as building the
gates themselves on host).

State index split: i = q5 * 128 + l7, with q5 = qubits 0-4 (5 MSBs) and
l7 = qubits 5-11 (7 LSBs, contiguous in memory -> 512B DMA bursts).
U_total = M5a (x) M7b with M5a = kron(G_0..G_4) [32x32] acting on q5 and
M7b = kron(G_5..G_11) [128x128] acting on l7.

Per-core pipeline (512 samples/core, 4 chunks of 128 samples b=(bh,b2),
bh in [0,32), b2 in [0,4); chunks split into 2 halves of 16 bh each,
each half = 4 groups of 4 c-tiles, c-tile = one bh = 4 samples):
  1. DMA-load half: Xh[(b2,q5), (bh,l7)] = x[bh*4+b2, q5*128+l7] (f32)
  2. cast Xh -> bf16 (VectorE 2x-pumped copy)
  3. stage 1 (PE "trick" matmuls, bf16): stationary = Xbf c-tile,
     moving = [Re(G5bd^T) | Im(G5bd^T)] with G5bd = I4 (x) M5a
     -> psum[l7, (j, re/im, (b2',q5'))] (applies the 5-qubit gate group
     AND transposes l7 onto partitions)
  4. evacuate psum: S1r = plain cast copy on ScalarE, S1i on VectorE
     (both bf16; no scaling -- x was pre-normalized on host)
  5. stage 2 (bf16): stationary = S1r/S1i c-tile chunks, moving =
     [Re(M7b^T)|Im(M7b^T)] / [-Im|Re], accumulating
     -> psum[(b2',q5'), (j, re/im, l7')]
  6. ONE ScalarE Square per group: T12 = psum^2 (f32, re and im halves)
  7. GpSimd adds re^2 + im^2 -> P
  8. DMA-store Ph[(b2,q5'), (bh, l7')] -> out[b, i]  (512B bursts)

Emission is software-pipelined (stage-2 of group g emitted after
stage-1 of group g+1) so the in-order Tensor queue never stalls on the
evacuation ops.
"""

from contextlib import ExitStack

import ml_dtypes
import numpy as np

import concourse.tile as tile
from concourse import bacc, mybir
from concourse.bass_utils import run_bass_kernel_spmd

F32 = mybir.dt.float32
F32R = mybir.dt.float32r
BF16 = mybir.dt.bfloat16

NUM_QUBITS = 12
D = 4096
B = 4096
N_CORES = 8
B_CORE = B // N_CORES  # 512
CHUNK = 128
N_CHUNKS = B_CORE // CHUNK  # 4
GROUP = 4  # c-tiles per psum group tile (2 banks)
HALF = D // 2  # free columns per half-chunk (16 bh x 128 l7)


def _u3(theta, phi, lam):
    """Single-qubit U3 gate, complex128 [2,2] (same formula as reference)."""
    c = np.cos(theta / 2.0)
    s = np.sin(theta / 2.0)
    return np.array(
        [
            [c, -np.exp(1j * lam) * s],
            [np.exp(1j * phi) * s, np.exp(1j * (phi + lam)) * c],
        ],
        dtype=np.complex128,
    )


def _gate_consts(thetas, phis, lams):
    """Build the constant moving-operand matrices for both PE stages (bf16)."""
    thetas = np.asarray(thetas, dtype=np.float64)
    phis = np.asarray(phis, dtype=np.float64)
    lams = np.asarray(lams, dtype=np.float64)
    gates = []
    for q in range(NUM_QUBITS):
        g1 = _u3(thetas[0, q], phis[0, q], lams[0, q])
        g2 = _u3(thetas[1, q], phis[1, q], lams[1, q])
        gates.append(g2 @ g1)  # layer 1 applied first, then layer 2

    m5a = gates[0]
    for q in range(1, 5):
        m5a = np.kron(m5a, gates[q])  # [32,32], acts on q5 (bits 0-4)
    m7b = gates[5]
    for q in range(6, 12):
        m7b = np.kron(m7b, gates[q])  # [128,128], acts on l7 (bits 5-11)

    g5 = np.kron(np.eye(4), m5a)  # [128,128] block-diag over (b2, q5)

    mv1 = np.concatenate([g5.T.real, g5.T.imag], axis=1)  # [128,256]
    mv2a = np.concatenate([m7b.T.real, m7b.T.imag], axis=1)
    mv2b = np.concatenate([-m7b.T.imag, m7b.T.real], axis=1)
    return (
        np.ascontiguousarray(mv1, dtype=np.float32),
        np.ascontiguousarray(mv2a).astype(ml_dtypes.bfloat16),
        np.ascontiguousarray(mv2b).astype(ml_dtypes.bfloat16),
    )


def _build_nc():
    nc = bacc.Bacc(
        "TRN2", target_bir_lowering=False, debug=False, num_devices=N_CORES
    )
    x_ap = nc.dram_tensor("x", [B_CORE, D], F32R, kind="ExternalInput").ap()
    mv1_ap = nc.dram_tensor("mv1", [128, 256], F32R, kind="ExternalInput").ap()
    mv2a_ap = nc.dram_tensor("mv2a", [128, 256], BF16, kind="ExternalInput").ap()
    mv2b_ap = nc.dram_tensor("mv2b", [128, 256], BF16, kind="ExternalInput").ap()
    out_ap = nc.dram_tensor("probs", [B_CORE, D], F32, kind="ExternalOutput").ap()

    with tile.TileContext(nc) as tc, ExitStack() as ctx:
        # gate constants FIRST on the sync queue so the pipeline can
        # start as soon as the first X pieces land
        consts = ctx.enter_context(tc.tile_pool(name="consts", bufs=1))
        mv1_tt = consts.tile([128, 256], F32R, tag="mv1")
        nc.sync.dma_start(mv1_tt[:], mv1_ap[:])
        mv2a_tt = consts.tile([128, 256], BF16, tag="mv2a")
        nc.sync.dma_start(mv2a_tt[:], mv2a_ap[:])
        mv2b_tt = consts.tile([128, 256], BF16, tag="mv2b")
        nc.sync.dma_start(mv2b_tt[:], mv2b_ap[:])
        mv1_t = mv1_tt[:]
        mv2a_t = mv2a_tt[:]
        mv2b_t = mv2b_tt[:]

        xpool0 = ctx.enter_context(tc.tile_pool(name="xp", bufs=8))
        all_Xh = [[None, None] for _ in range(N_CHUNKS)]

        def emit_load(k, eng=None):
            eng = eng or nc.sync
            xflat = x_ap[k * CHUNK : (k + 1) * CHUNK, :].flatten()
            QTR = HALF // 2
            for h in range(2):
                X = xpool0.tile([128, HALF], F32R, tag="X")
                all_Xh[k][h] = X
                for q in range(2):
                    eng.dma_start(
                        X[:, q * QTR : (q + 1) * QTR].rearrange(
                            "p (bh l) -> p bh l", l=128
                        ),
                        xflat[
                            (2 * h + q) * CHUNK * QTR : (2 * h + q + 1) * CHUNK * QTR
                        ].rearrange("(bh p l) -> p bh l", p=128, l=128),
                    )

        # chunk 0, half 0 in eighth-granularity so the first stage-1 group
        # (bh 0-3) can start as early as possible
        xflat0 = x_ap[0:CHUNK, :].flatten()
        EGT = HALF // 4
        X00 = xpool0.tile([128, HALF], F32R, tag="X")
        all_Xh[0][0] = X00
        for e in range(4):
            nc.sync.dma_start(
                X00[:, e * EGT : (e + 1) * EGT].rearrange(
                    "p (bh l) -> p bh l", l=128
                ),
                xflat0[e * CHUNK * EGT : (e + 1) * CHUNK * EGT].rearrange(
                    "(bh p l) -> p bh l", p=128, l=128
                ),
            )
        QTR0 = HALF // 2
        X01 = xpool0.tile([128, HALF], F32R, tag="X")
        all_Xh[0][1] = X01
        for q in range(2):
            nc.sync.dma_start(
                X01[:, q * QTR0 : (q + 1) * QTR0].rearrange(
                    "p (bh l) -> p bh l", l=128
                ),
                xflat0[
                    (2 + q) * CHUNK * QTR0 : (3 + q) * CHUNK * QTR0
                ].rearrange("(bh p l) -> p bh l", p=128, l=128),
            )
        for k in range(1, N_CHUNKS):
            emit_load(k)

        s1pool = ctx.enter_context(tc.tile_pool(name="s1p", bufs=6))
        t12p = ctx.enter_context(tc.tile_pool(name="t12", bufs=4))
        ppool = ctx.enter_context(tc.tile_pool(name="pp", bufs=4))
        ps1 = ctx.enter_context(tc.tile_pool(name="ps1", bufs=2, space="PSUM"))
        ps2 = ctx.enter_context(tc.tile_pool(name="ps2", bufs=2, space="PSUM"))

        # ---- software-pipelined group stream --------------------------
        # One "item" = (k, h, gl). Stage-2+square of item g is emitted
        # after stage-1 of item g+1; adds+store happen per half.
        items = [
            (k, h, gl)
            for k in range(N_CHUNKS)
            for h in range(2)
            for gl in range(4)
        ]
        state = {}  # (k,h) -> dict with T12 tile
        pend = {}  # item -> S1ri tile
        LAST = items[-1][:2]

        def emit_half_prologue(k, h):
            T12 = t12p.tile([128, 2 * HALF], BF16, tag="T12")
            P = ppool.tile([128, HALF], F32, tag="P")
            state[(k, h)] = {"T12": T12, "P": P}

        def emit_stage1(item):
            k, h, gl = item
            X = all_Xh[k][h]
            pg = ps1.tile([128, GROUP * 256], F32, tag="g1")
            for j in range(GROUP):
                cl = gl * GROUP + j
                nc.tensor.matmul(
                    pg[:, j * 256 : (j + 1) * 256],
                    lhsT=X[:, cl * 128 : (cl + 1) * 128],
                    rhs=mv1_t,
                    start=True,
                    stop=True,
                )
            # evacuate: ONE contiguous bf16 cast on VectorE, keeping pg's
            # (j, re/im, q) layout so stage-2 lhsT slices stay contiguous
            S1ri = s1pool.tile([128, GROUP * 256], BF16, tag="S1ri")
            nc.vector.tensor_copy(S1ri[:], pg[:])
            return S1ri

        def emit_stage2(item, S1ri):
            k, h, gl = item
            st = state[(k, h)]
            pg2 = ps2.tile([128, GROUP * 256], F32, tag="g2")
            for j in range(GROUP):
                nc.tensor.matmul(
                    pg2[:, j * 256 : (j + 1) * 256],
                    lhsT=S1ri[:, j * 256 : j * 256 + 128],
                    rhs=mv2a_t,
                    start=True,
                    stop=False,
                )
                nc.tensor.matmul(
                    pg2[:, j * 256 : (j + 1) * 256],
                    lhsT=S1ri[:, j * 256 + 128 : (j + 1) * 256],
                    rhs=mv2b_t,
                    start=False,
                    stop=True,
                )
            # both squares (re and im) in one ScalarE op
            gcols = slice(gl * GROUP * 256, (gl + 1) * GROUP * 256)
            nc.scalar.square(st["T12"][:, gcols], pg2[:])
            # per-group add + store: spreads store DMA traffic across
            # the whole kernel instead of bunching it at half boundaries
            last = (k, h) == LAST
            T5 = st["T12"][:].rearrange(
                "p (g j c q) -> p g j c q", g=4, j=GROUP, c=2
            )
            P = st["P"]
            P4 = P[:].rearrange("p (g j q) -> p g j q", g=4, j=GROUP)
            # adds on GpSimd (otherwise idle); for the final half,
            # alternate VectorE/GpSimd so the tail drains in parallel
            eng = (nc.vector if gl % 2 == 0 else nc.gpsimd) if last else nc.gpsimd
            eng.tensor_tensor(
                P4[:, gl],
                T5[:, gl, :, 0],
                T5[:, gl, :, 1],
                op=mybir.AluOpType.add,
            )
            oflat = out_ap[k * CHUNK : (k + 1) * CHUNK, :].flatten()
            base = h * CHUNK * HALF
            QC = GROUP * 128  # columns per group quarter
            # route chunks 0-1 to the scalar HWDGE ring (sync is busy
            # issuing loads early on), chunks 2-3 to sync
            qeng = nc.scalar if k < 2 else nc.sync
            qb = base + gl * CHUNK * QC
            qeng.dma_start(
                oflat[qb : qb + CHUNK * QC].rearrange(
                    "(bh p l) -> p bh l", p=128, l=128
                ),
                P[:, gl * QC : (gl + 1) * QC].rearrange(
                    "p (bh l) -> p bh l", l=128
                ),
            )
            if gl == 3:
                state.pop((k, h))

        # depth-2 software pipeline: stage-2 of item i runs after
        # stage-1 of item i+2, giving the evacuation two groups of
        # tensor time to complete
        for idx, item in enumerate(items):
            k, h, gl = item
            if gl == 0:
                emit_half_prologue(k, h)
            pend[item] = emit_stage1(item)

            if idx >= 2:
                p2 = items[idx - 2]
                emit_stage2(p2, pend.pop(p2))
        for p2 in items[-2:]:
            emit_stage2(p2, pend.pop(p2))

    nc.compile()
    return nc


_NC_CACHE = {}


def _get_nc():
    if "nc" not in _NC_CACHE:
        _NC_CACHE["nc"] = _build_nc()
    return _NC_CACHE["nc"]


def kernel(inputs, thetas, phis, lams, _trace=False, _trace_kwargs=None):
    inputs = np.ascontiguousarray(np.asarray(inputs), dtype=np.float32)
    # host-side pre-normalization (per-sample L2 norm), f64 accumulate
    nrm = np.sqrt(
        np.einsum("bi,bi->b", inputs, inputs, dtype=np.float64)
    ).astype(np.float32)
    xn = inputs / nrm[:, None]
    mv1, mv2a, mv2b = _gate_consts(thetas, phis, lams)

    nc = _get_nc()
    in_maps = [
        {
            "x": xn[k * B_CORE : (k + 1) * B_CORE],
            "mv1": mv1,
            "mv2a": mv2a,
            "mv2b": mv2b,
        }
        for k in range(N_CORES)
    ]
    res = run_bass_kernel_spmd(
        nc, in_maps, list(range(N_CORES)), trace=_trace, **(_trace_kwargs or {})
    )
    out = np.concatenate([res.results[k]["probs"] for k in range(N_CORES)], axis=0)
    if _trace:
        kernel.last_result = res
    return out


# revision 20
# speedup vs baseline: 1.0896x; 1.0896x over previous
"""Trainium2 Bass kernel for the DifferentiableQuantumCircuit problem.

Math: output = |U x / ||x|| |^2 with U = kron of 12 single-qubit U3 gates
applied twice (2 layers). Gates on different qubits commute, so the two
layers fuse into ONE kron-product unitary with per-qubit gates
G_q = U3_layer2(q) @ U3_layer1(q).

Host side: x is pre-normalized (x / ||x||, numpy) and the gate constants
are built in float64 then cast to bf16 (same precedent as building the
gates themselves on host).

State index split: i = q5 * 128 + l7, with q5 = qubits 0-4 (5 MSBs) and
l7 = qubits 5-11 (7 LSBs, contiguous in memory -> 512B DMA bursts).
U_total = M5a (x) M7b with M5a = kron(G_0..G_4) [32x32] acting on q5 and
M7b = kron(G_5..G_11) [128x128] acting on l7.

Per-core pipeline (512 samples/core, 4 chunks of 128 samples b=(bh,b2),
bh in [0,32), b2 in [0,4); chunks split into 2 halves of 16 bh each,
each half = 4 groups of 4 c-tiles, c-tile = one bh = 4 samples):
  1. DMA-load half: Xh[(b2,q5), (bh,l7)] = x[bh*4+b2, q5*128+l7] (f32)
  2. cast Xh -> bf16 (VectorE 2x-pumped copy)
  3. stage 1 (PE "trick" matmuls, bf16): stationary = Xbf c-tile,
     moving = [Re(G5bd^T) | Im(G5bd^T)] with G5bd = I4 (x) M5a
     -> psum[l7, (j, re/im, (b2',q5'))] (applies the 5-qubit gate group
     AND transposes l7 onto partitions)
  4. evacuate psum: S1r = plain cast copy on ScalarE, S1i on VectorE
     (both bf16; no scaling -- x was pre-normalized on host)
  5. stage 2 (bf16): stationary = S1r/S1i c-tile chunks, moving =
     [Re(M7b^T)|Im(M7b^T)] / [-Im|Re], accumulating
     -> psum[(b2',q5'), (j, re/im, l7')]
  6. ONE ScalarE Square per group: T12 = psum^2 (f32, re and im halves)
  7. GpSimd adds re^2 + im^2 -> P
  8. DMA-store Ph[(b2,q5'), (bh, l7')] -> out[b, i]  (512B bursts)

Emission is software-pipelined (stage-2 of group g emitted after
stage-1 of group g+1) so the in-order Tensor queue never stalls on the
evacuation ops.
"""

from contextlib import ExitStack

import ml_dtypes
import numpy as np

import concourse.tile as tile
from concourse import bacc, mybir
from concourse.bass_utils import run_bass_kernel_spmd

F32 = mybir.dt.float32
F32R = mybir.dt.float32r
BF16 = mybir.dt.bfloat16

NUM_QUBITS = 12
D = 4096
B = 4096
N_CORES = 8
B_CORE = B // N_CORES  # 512
CHUNK = 128
N_CHUNKS = B_CORE // CHUNK  # 4
GROUP = 4  # c-tiles per psum group tile (2 banks)
HALF = D // 2  # free columns per half-chunk (16 bh x 128 l7)


def _u3(theta, phi, lam):
    """Single-qubit U3 gate, complex128 [2,2] (same formula as reference)."""
    c = np.cos(theta / 2.0)
    s = np.sin(theta / 2.0)
    return np.array(
        [
            [c, -np.exp(1j * lam) * s],
            [np.exp(1j * phi) * s, np.exp(1j * (phi + lam)) * c],
        ],
        dtype=np.complex128,
    )


def _gate_consts(thetas, phis, lams):
    """Build the constant moving-operand matrices for both PE stages (bf16)."""
    thetas = np.asarray(thetas, dtype=np.float64)
    phis = np.asarray(phis, dtype=np.float64)
    lams = np.asarray(lams, dtype=np.float64)
    gates = []
    for q in range(NUM_QUBITS):
        g1 = _u3(thetas[0, q], phis[0, q], lams[0, q])
        g2 = _u3(thetas[1, q], phis[1, q], lams[1, q])
        gates.append(g2 @ g1)  # layer 1 applied first, then layer 2

    m5a = gates[0]
    for q in range(1, 5):
        m5a = np.kron(m5a, gates[q])  # [32,32], acts on q5 (bits 0-4)
    m7b = gates[5]
    for q in range(6, 12):
        m7b = np.kron(m7b, gates[q])  # [128,128], acts on l7 (bits 5-11)

    g5 = np.kron(np.eye(4), m5a)  # [128,128] block-diag over (b2, q5)

    mv1 = np.concatenate([g5.T.real, g5.T.imag], axis=1)  # [128,256]
    mv2a = np.concatenate([m7b.T.real, m7b.T.imag], axis=1)
    mv2b = np.concatenate([-m7b.T.imag, m7b.T.real], axis=1)
    return (
        np.ascontiguousarray(mv1, dtype=np.float32),
        np.ascontiguousarray(mv2a).astype(ml_dtypes.bfloat16),
        np.ascontiguousarray(mv2b).astype(ml_dtypes.bfloat16),
    )


def _build_nc():
    nc = bacc.Bacc(
        "TRN2", target_bir_lowering=False, debug=False, num_devices=N_CORES
    )
    x_ap = nc.dram_tensor("x", [B_CORE, D], F32R, kind="ExternalInput").ap()
    mv1_ap = nc.dram_tensor("mv1", [128, 256], F32R, kind="ExternalInput").ap()
    mv2a_ap = nc.dram_tensor("mv2a", [128, 256], BF16, kind="ExternalInput").ap()
    mv2b_ap = nc.dram_tensor("mv2b", [128, 256], BF16, kind="ExternalInput").ap()
    out_ap = nc.dram_tensor("probs", [B_CORE, D], F32, kind="ExternalOutput").ap()

    with tile.TileContext(nc) as tc, ExitStack() as ctx:
        # gate constants FIRST on the sync queue so the pipeline can
        # start as soon as the first X pieces land
        consts = ctx.enter_context(tc.tile_pool(name="consts", bufs=1))
        mv1_tt = consts.tile([128, 256], F32R, tag="mv1")
        nc.sync.dma_start(mv1_tt[:], mv1_ap[:])
        mv2a_tt = consts.tile([128, 256], BF16, tag="mv2a")
        nc.sync.dma_start(mv2a_tt[:], mv2a_ap[:])
        mv2b_tt = consts.tile([128, 256], BF16, tag="mv2b")
        nc.sync.dma_start(mv2b_tt[:], mv2b_ap[:])
        mv1_t = mv1_tt[:]
        mv2a_t = mv2a_tt[:]
        mv2b_t = mv2b_tt[:]

        xpool0 = ctx.enter_context(tc.tile_pool(name="xp", bufs=8))
        all_Xh = [[None, None] for _ in range(N_CHUNKS)]

        def emit_load(k, eng=None):
            eng = eng or nc.sync
            xflat = x_ap[k * CHUNK : (k + 1) * CHUNK, :].flatten()
            QTR = HALF // 2
            for h in range(2):
                X = xpool0.tile([128, HALF], F32R, tag="X")
                all_Xh[k][h] = X
                for q in range(2):
                    eng.dma_start(
                        X[:, q * QTR : (q + 1) * QTR].rearrange(
                            "p (bh l) -> p bh l", l=128
                        ),
                        xflat[
                            (2 * h + q) * CHUNK * QTR : (2 * h + q + 1) * CHUNK * QTR
                        ].rearrange("(bh p l) -> p bh l", p=128, l=128),
                    )

        # chunk 0, half 0 in eighth-granularity so the first stage-1 group
        # (bh 0-3) can start as early as possible
        xflat0 = x_ap[0:CHUNK, :].flatten()
        EGT = HALF // 4
        X00 = xpool0.tile([128, HALF], F32R, tag="X")
        all_Xh[0][0] = X00
        for e in range(4):
            nc.sync.dma_start(
                X00[:, e * EGT : (e + 1) * EGT].rearrange(
                    "p (bh l) -> p bh l", l=128
                ),
                xflat0[e * CHUNK * EGT : (e + 1) * CHUNK * EGT].rearrange(
                    "(bh p l) -> p bh l", p=128, l=128
                ),
            )
        QTR0 = HALF // 2
        X01 = xpool0.tile([128, HALF], F32R, tag="X")
        all_Xh[0][1] = X01
        for q in range(2):
            nc.sync.dma_start(
                X01[:, q * QTR0 : (q + 1) * QTR0].rearrange(
                    "p (bh l) -> p bh l", l=128
                ),
                xflat0[
                    (2 + q) * CHUNK * QTR0 : (3 + q) * CHUNK * QTR0
                ].rearrange("(bh p l) -> p bh l", p=128, l=128),
            )
        for k in range(1, N_CHUNKS):
            emit_load(k)

        s1pool = ctx.enter_context(tc.tile_pool(name="s1p", bufs=6))
        t12p = ctx.enter_context(tc.tile_pool(name="t12", bufs=4))
        ppool = ctx.enter_context(tc.tile_pool(name="pp", bufs=4))
        ps1 = ctx.enter_context(tc.tile_pool(name="ps1", bufs=2, space="PSUM"))
        ps2 = ctx.enter_context(tc.tile_pool(name="ps2", bufs=2, space="PSUM"))

        # ---- software-pipelined group stream --------------------------
        # One "item" = (k, h, gl). Stage-2+square of item g is emitted
        # after stage-1 of item g+1; adds+store happen per half.
        items = [
            (k, h, gl)
            for k in range(N_CHUNKS)
            for h in range(2)
            for gl in range(4)
        ]
        state = {}  # (k,h) -> dict with T12 tile
        pend = {}  # item -> S1ri tile
        pending_stores = []  # deferred store closures
        LAST = items[-1][:2]

        def emit_half_prologue(k, h):
            T12 = t12p.tile([128, 2 * HALF], BF16, tag="T12")
            P = ppool.tile([128, HALF], F32, tag="P")
            state[(k, h)] = {"T12": T12, "P": P}

        def emit_stage1(item):
            k, h, gl = item
            X = all_Xh[k][h]
            pg = ps1.tile([128, GROUP * 256], F32, tag="g1")
            for j in range(GROUP):
                cl = gl * GROUP + j
                nc.tensor.matmul(
                    pg[:, j * 256 : (j + 1) * 256],
                    lhsT=X[:, cl * 128 : (cl + 1) * 128],
                    rhs=mv1_t,
                    start=True,
                    stop=True,
                )
            # evacuate: ONE contiguous bf16 cast on VectorE, keeping pg's
            # (j, re/im, q) layout so stage-2 lhsT slices stay contiguous
            S1ri = s1pool.tile([128, GROUP * 256], BF16, tag="S1ri")
            nc.vector.tensor_copy(S1ri[:], pg[:])
            return S1ri

        def emit_stage2(item, S1ri):
            k, h, gl = item
            st = state[(k, h)]
            pg2 = ps2.tile([128, GROUP * 256], F32, tag="g2")
            for j in range(GROUP):
                nc.tensor.matmul(
                    pg2[:, j * 256 : (j + 1) * 256],
                    lhsT=S1ri[:, j * 256 : j * 256 + 128],
                    rhs=mv2a_t,
                    start=True,
                    stop=False,
                )
                nc.tensor.matmul(
                    pg2[:, j * 256 : (j + 1) * 256],
                    lhsT=S1ri[:, j * 256 + 128 : (j + 1) * 256],
                    rhs=mv2b_t,
                    start=False,
                    stop=True,
                )
            # both squares (re and im) in one ScalarE op
            gcols = slice(gl * GROUP * 256, (gl + 1) * GROUP * 256)
            nc.scalar.square(st["T12"][:, gcols], pg2[:])
            # per-group add + store: spreads store DMA traffic across
            # the whole kernel instead of bunching it at half boundaries
            last = (k, h) == LAST
            T5 = st["T12"][:].rearrange(
                "p (g j c q) -> p g j c q", g=4, j=GROUP, c=2
            )
            P = st["P"]
            P4 = P[:].rearrange("p (g j q) -> p g j q", g=4, j=GROUP)
            # adds on GpSimd (otherwise idle); for the final half,
            # alternate VectorE/GpSimd so the tail drains in parallel
            eng = (nc.vector if gl % 2 == 0 else nc.gpsimd) if last else nc.gpsimd
            eng.tensor_tensor(
                P4[:, gl],
                T5[:, gl, :, 0],
                T5[:, gl, :, 1],
                op=mybir.AluOpType.add,
            )
            oflat = out_ap[k * CHUNK : (k + 1) * CHUNK, :].flatten()
            base = h * CHUNK * HALF
            QC = GROUP * 128  # columns per group quarter
            # route chunks 0-1 to the scalar HWDGE ring (sync is busy
            # issuing loads early on), chunks 2-3 to sync
            qeng = nc.scalar if k < 2 else nc.sync
            qb = base + gl * CHUNK * QC

            def _store(qeng=qeng, qb=qb, P=P, gl=gl, oflat=oflat):
                qeng.dma_start(
                    oflat[qb : qb + CHUNK * QC].rearrange(
                        "(bh p l) -> p bh l", p=128, l=128
                    ),
                    P[:, gl * QC : (gl + 1) * QC].rearrange(
                        "p (bh l) -> p bh l", l=128
                    ),
                )

            # defer the store issue by 2 groups so the add it depends on
            # has finished by the time it reaches the queue head
            pending_stores.append(_store)
            if len(pending_stores) > 2:
                pending_stores.pop(0)()
            if gl == 3:
                state.pop((k, h))

        # depth-2 software pipeline: stage-2 of item i runs after
        # stage-1 of item i+2, giving the evacuation two groups of
        # tensor time to complete
        for idx, item in enumerate(items):
            k, h, gl = item
            if gl == 0:
                emit_half_prologue(k, h)
            pend[item] = emit_stage1(item)

            if idx >= 2:
                p2 = items[idx - 2]
                emit_stage2(p2, pend.pop(p2))
        for p2 in items[-2:]:
            emit_stage2(p2, pend.pop(p2))
        for s in pending_stores:
            s()

    nc.compile()
    return nc


_NC_CACHE = {}


def _get_nc():
    if "nc" not in _NC_CACHE:
        _NC_CACHE["nc"] = _build_nc()
    return _NC_CACHE["nc"]


def kernel(inputs, thetas, phis, lams, _trace=False, _trace_kwargs=None):
    inputs = np.ascontiguousarray(np.asarray(inputs), dtype=np.float32)
    # host-side pre-normalization (per-sample L2 norm), f64 accumulate
    nrm = np.sqrt(
        np.einsum("bi,bi->b", inputs, inputs, dtype=np.float64)
    ).astype(np.float32)
    xn = inputs / nrm[:, None]
    mv1, mv2a, mv2b = _gate_consts(thetas, phis, lams)

    nc = _get_nc()
    in_maps = [
        {
            "x": xn[k * B_CORE : (k + 1) * B_CORE],
            "mv1": mv1,
            "mv2a": mv2a,
            "mv2b": mv2b,
        }
        for k in range(N_CORES)
    ]
    res = run_bass_kernel_spmd(
        nc, in_maps, list(range(N_CORES)), trace=_trace, **(_trace_kwargs or {})
    )
    out = np.concatenate([res.results[k]["probs"] for k in range(N_CORES)], axis=0)
    if _trace:
        kernel.last_result = res
    return out


# revision 22
# speedup vs baseline: 1.2116x; 1.1119x over previous
"""Trainium2 Bass kernel for the DifferentiableQuantumCircuit problem.

Math: output = |U x / ||x|| |^2 with U = kron of 12 single-qubit U3 gates
applied twice (2 layers). Gates on different qubits commute, so the two
layers fuse into ONE kron-product unitary with per-qubit gates
G_q = U3_layer2(q) @ U3_layer1(q).

Host side: x is pre-normalized (x / ||x||, numpy) and the gate constants
are built in float64 then cast to bf16 (same precedent as building the
gates themselves on host).

State index split: i = q5 * 128 + l7, with q5 = qubits 0-4 (5 MSBs) and
l7 = qubits 5-11 (7 LSBs, contiguous in memory -> 512B DMA bursts).
U_total = M5a (x) M7b with M5a = kron(G_0..G_4) [32x32] acting on q5 and
M7b = kron(G_5..G_11) [128x128] acting on l7.

Per-core pipeline (512 samples/core, 4 chunks of 128 samples b=(bh,b2),
bh in [0,32), b2 in [0,4); chunks split into 2 halves of 16 bh each,
each half = 4 groups of 4 c-tiles, c-tile = one bh = 4 samples):
  1. DMA-load half: Xh[(b2,q5), (bh,l7)] = x[bh*4+b2, q5*128+l7] (f32)
  2. cast Xh -> bf16 (VectorE 2x-pumped copy)
  3. stage 1 (PE "trick" matmuls, bf16): stationary = Xbf c-tile,
     moving = [Re(G5bd^T) | Im(G5bd^T)] with G5bd = I4 (x) M5a
     -> psum[l7, (j, re/im, (b2',q5'))] (applies the 5-qubit gate group
     AND transposes l7 onto partitions)
  4. evacuate psum: S1r = plain cast copy on ScalarE, S1i on VectorE
     (both bf16; no scaling -- x was pre-normalized on host)
  5. stage 2 (bf16): stationary = S1r/S1i c-tile chunks, moving =
     [Re(M7b^T)|Im(M7b^T)] / [-Im|Re], accumulating
     -> psum[(b2',q5'), (j, re/im, l7')]
  6. ONE ScalarE Square per group: T12 = psum^2 (f32, re and im halves)
  7. GpSimd adds re^2 + im^2 -> P
  8. DMA-store Ph[(b2,q5'), (bh, l7')] -> out[b, i]  (512B bursts)

Emission is software-pipelined (stage-2 of group g emitted after
stage-1 of group g+1) so the in-order Tensor queue never stalls on the
evacuation ops.
"""

from contextlib import ExitStack

import ml_dtypes
import numpy as np

import concourse.tile as tile
from concourse import bacc, mybir
from concourse.bass_utils import run_bass_kernel_spmd

F32 = mybir.dt.float32
F32R = mybir.dt.float32r
BF16 = mybir.dt.bfloat16

NUM_QUBITS = 12
D = 4096
B = 4096
N_CORES = 8
B_CORE = B // N_CORES  # 512
CHUNK = 128
N_CHUNKS = B_CORE // CHUNK  # 4
GROUP = 4  # c-tiles per psum group tile (2 banks)
HALF = D // 2  # free columns per half-chunk (16 bh x 128 l7)


def _u3(theta, phi, lam):
    """Single-qubit U3 gate, complex128 [2,2] (same formula as reference)."""
    c = np.cos(theta / 2.0)
    s = np.sin(theta / 2.0)
    return np.array(
        [
            [c, -np.exp(1j * lam) * s],
            [np.exp(1j * phi) * s, np.exp(1j * (phi + lam)) * c],
        ],
        dtype=np.complex128,
    )


def _gate_consts(thetas, phis, lams):
    """Build the constant moving-operand matrices for both PE stages (bf16)."""
    thetas = np.asarray(thetas, dtype=np.float64)
    phis = np.asarray(phis, dtype=np.float64)
    lams = np.asarray(lams, dtype=np.float64)
    gates = []
    for q in range(NUM_QUBITS):
        g1 = _u3(thetas[0, q], phis[0, q], lams[0, q])
        g2 = _u3(thetas[1, q], phis[1, q], lams[1, q])
        gates.append(g2 @ g1)  # layer 1 applied first, then layer 2

    m5a = gates[0]
    for q in range(1, 5):
        m5a = np.kron(m5a, gates[q])  # [32,32], acts on q5 (bits 0-4)
    m7b = gates[5]
    for q in range(6, 12):
        m7b = np.kron(m7b, gates[q])  # [128,128], acts on l7 (bits 5-11)

    g5 = np.kron(np.eye(4), m5a)  # [128,128] block-diag over (b2, q5)

    mv1 = np.concatenate([g5.T.real, g5.T.imag], axis=1)  # [128,256]
    mv2a = np.concatenate([m7b.T.real, m7b.T.imag], axis=1)
    mv2b = np.concatenate([-m7b.T.imag, m7b.T.real], axis=1)
    return (
        np.ascontiguousarray(mv1, dtype=np.float32),
        np.ascontiguousarray(mv2a).astype(ml_dtypes.bfloat16),
        np.ascontiguousarray(mv2b).astype(ml_dtypes.bfloat16),
    )


def _build_nc():
    nc = bacc.Bacc(
        "TRN2", target_bir_lowering=False, debug=False, num_devices=N_CORES
    )
    x_ap = nc.dram_tensor("x", [B_CORE, D], F32R, kind="ExternalInput").ap()
    mv1_ap = nc.dram_tensor("mv1", [128, 256], F32R, kind="ExternalInput").ap()
    mv2a_ap = nc.dram_tensor("mv2a", [128, 256], BF16, kind="ExternalInput").ap()
    mv2b_ap = nc.dram_tensor("mv2b", [128, 256], BF16, kind="ExternalInput").ap()
    out_ap = nc.dram_tensor("probs", [B_CORE, D], F32, kind="ExternalOutput").ap()

    with tile.TileContext(nc) as tc, ExitStack() as ctx:
        # gate constants FIRST on the sync queue so the pipeline can
        # start as soon as the first X pieces land
        consts = ctx.enter_context(tc.tile_pool(name="consts", bufs=1))
        mv1_tt = consts.tile([128, 256], F32R, tag="mv1")
        nc.sync.dma_start(mv1_tt[:], mv1_ap[:])
        mv2a_tt = consts.tile([128, 256], BF16, tag="mv2a")
        nc.sync.dma_start(mv2a_tt[:], mv2a_ap[:])
        mv2b_tt = consts.tile([128, 256], BF16, tag="mv2b")
        nc.sync.dma_start(mv2b_tt[:], mv2b_ap[:])
        mv1_t = mv1_tt[:]
        mv2a_t = mv2a_tt[:]
        mv2b_t = mv2b_tt[:]

        xpool0 = ctx.enter_context(tc.tile_pool(name="xp", bufs=8))
        all_Xh = [[None, None] for _ in range(N_CHUNKS)]

        def emit_load(k, eng=None):
            eng = eng or nc.sync
            xflat = x_ap[k * CHUNK : (k + 1) * CHUNK, :].flatten()
            QTR = HALF // 2
            for h in range(2):
                X = xpool0.tile([128, HALF], F32R, tag="X")
                all_Xh[k][h] = X
                for q in range(2):
                    eng.dma_start(
                        X[:, q * QTR : (q + 1) * QTR].rearrange(
                            "p (bh l) -> p bh l", l=128
                        ),
                        xflat[
                            (2 * h + q) * CHUNK * QTR : (2 * h + q + 1) * CHUNK * QTR
                        ].rearrange("(bh p l) -> p bh l", p=128, l=128),
                    )

        # chunk 0, half 0 in eighth-granularity so the first stage-1 group
        # (bh 0-3) can start as early as possible
        xflat0 = x_ap[0:CHUNK, :].flatten()
        EGT = HALF // 4
        X00 = xpool0.tile([128, HALF], F32R, tag="X")
        all_Xh[0][0] = X00
        for e in range(4):
            nc.sync.dma_start(
                X00[:, e * EGT : (e + 1) * EGT].rearrange(
                    "p (bh l) -> p bh l", l=128
                ),
                xflat0[e * CHUNK * EGT : (e + 1) * CHUNK * EGT].rearrange(
                    "(bh p l) -> p bh l", p=128, l=128
                ),
            )
        QTR0 = HALF // 2
        X01 = xpool0.tile([128, HALF], F32R, tag="X")
        all_Xh[0][1] = X01
        for q in range(2):
            nc.sync.dma_start(
                X01[:, q * QTR0 : (q + 1) * QTR0].rearrange(
                    "p (bh l) -> p bh l", l=128
                ),
                xflat0[
                    (2 + q) * CHUNK * QTR0 : (3 + q) * CHUNK * QTR0
                ].rearrange("(bh p l) -> p bh l", p=128, l=128),
            )
        for k in range(1, N_CHUNKS):
            emit_load(k)

        s1pool = ctx.enter_context(tc.tile_pool(name="s1p", bufs=6))
        t12p = ctx.enter_context(tc.tile_pool(name="t12", bufs=4))
        ppool = ctx.enter_context(tc.tile_pool(name="pp", bufs=4))
        ps1 = ctx.enter_context(tc.tile_pool(name="ps1", bufs=2, space="PSUM"))
        ps2 = ctx.enter_context(tc.tile_pool(name="ps2", bufs=2, space="PSUM"))

        # ---- software-pipelined group stream --------------------------
        # One "item" = (k, h, gl). Stage-2+square of item g is emitted
        # after stage-1 of item g+1; adds+store happen per half.
        items = [
            (k, h, gl)
            for k in range(N_CHUNKS)
            for h in range(2)
            for gl in range(4)
        ]
        state = {}  # (k,h) -> dict with T12 tile
        pend = {}  # item -> S1ri tile
        pending_stores = []  # deferred store closures
        LAST = items[-1][:2]

        def emit_half_prologue(k, h):
            T12 = t12p.tile([128, 2 * HALF], BF16, tag="T12")
            P = ppool.tile([128, HALF], F32, tag="P")
            state[(k, h)] = {"T12": T12, "P": P}

        def emit_stage1(item):
            k, h, gl = item
            X = all_Xh[k][h]
            pg = ps1.tile([128, GROUP * 256], F32, tag="g1")
            for j in range(GROUP):
                cl = gl * GROUP + j
                nc.tensor.matmul(
                    pg[:, j * 256 : (j + 1) * 256],
                    lhsT=X[:, cl * 128 : (cl + 1) * 128],
                    rhs=mv1_t,
                    start=True,
                    stop=True,
                )
            # evacuate: ONE contiguous bf16 cast on VectorE, keeping pg's
            # (j, re/im, q) layout so stage-2 lhsT slices stay contiguous
            S1ri = s1pool.tile([128, GROUP * 256], BF16, tag="S1ri")
            nc.vector.tensor_copy(S1ri[:], pg[:])
            return S1ri

        def emit_stage2(item, S1ri):
            k, h, gl = item
            st = state[(k, h)]
            pg2 = ps2.tile([128, GROUP * 256], F32, tag="g2")
            for j in range(GROUP):
                nc.tensor.matmul(
                    pg2[:, j * 256 : (j + 1) * 256],
                    lhsT=S1ri[:, j * 256 : j * 256 + 128],
                    rhs=mv2a_t,
                    start=True,
                    stop=False,
                )
                nc.tensor.matmul(
                    pg2[:, j * 256 : (j + 1) * 256],
                    lhsT=S1ri[:, j * 256 + 128 : (j + 1) * 256],
                    rhs=mv2b_t,
                    start=False,
                    stop=True,
                )
            # both squares (re and im) in one ScalarE op
            gcols = slice(gl * GROUP * 256, (gl + 1) * GROUP * 256)
            nc.scalar.square(st["T12"][:, gcols], pg2[:])

        def emit_half_epilogue(k, h):
            st = state.pop((k, h))
            T12 = st["T12"]
            last = (k, h) == LAST
            P = st["P"]
            T5 = T12[:].rearrange(
                "p (g j c q) -> p g j c q", g=4, j=GROUP, c=2
            )
            P4 = P[:].rearrange("p (g j q) -> p g j q", g=4, j=GROUP)
            oflat = out_ap[k * CHUNK : (k + 1) * CHUNK, :].flatten()
            base = h * CHUNK * HALF
            QC = GROUP * 128  # columns per group quarter
            for gl in range(4):
                # adds on GpSimd (otherwise idle); for the final half,
                # alternate so the tail drains in parallel
                eng = (nc.vector if gl % 2 == 0 else nc.gpsimd) if last else nc.gpsimd
                eng.tensor_tensor(
                    P4[:, gl],
                    T5[:, gl, :, 0],
                    T5[:, gl, :, 1],
                    op=mybir.AluOpType.add,
                )
                # route chunks 0-1 to the scalar HWDGE ring (sync is
                # busy issuing loads early on), chunks 2-3 to sync
                qeng = nc.scalar if k < 2 else nc.sync
                qb = base + gl * CHUNK * QC

                def _store(qeng=qeng, qb=qb, P=P, gl=gl, oflat=oflat):
                    qeng.dma_start(
                        oflat[qb : qb + CHUNK * QC].rearrange(
                            "(bh p l) -> p bh l", p=128, l=128
                        ),
                        P[:, gl * QC : (gl + 1) * QC].rearrange(
                            "p (bh l) -> p bh l", l=128
                        ),
                    )

                # defer store issue so the add it depends on has
                # finished by the time it reaches the queue head
                pending_stores.append(_store)
                if len(pending_stores) > 3:
                    pending_stores.pop(0)()

        # depth-2 software pipeline: stage-2 of item i runs after
        # stage-1 of item i+2, giving the evacuation two groups of
        # tensor time to complete
        for idx, item in enumerate(items):
            k, h, gl = item
            if gl == 0:
                emit_half_prologue(k, h)
            pend[item] = emit_stage1(item)

            if idx >= 2:
                p2 = items[idx - 2]
                emit_stage2(p2, pend.pop(p2))
                if p2[2] == 3:
                    emit_half_epilogue(p2[0], p2[1])
        for p2 in items[-2:]:
            emit_stage2(p2, pend.pop(p2))
            if p2[2] == 3:
                emit_half_epilogue(p2[0], p2[1])
        for s in pending_stores:
            s()

    nc.compile()
    return nc


_NC_CACHE = {}


def _get_nc():
    if "nc" not in _NC_CACHE:
        _NC_CACHE["nc"] = _build_nc()
    return _NC_CACHE["nc"]


def kernel(inputs, thetas, phis, lams, _trace=False, _trace_kwargs=None):
    inputs = np.ascontiguousarray(np.asarray(inputs), dtype=np.float32)
    # host-side pre-normalization (per-sample L2 norm), f64 accumulate
    nrm = np.sqrt(
        np.einsum("bi,bi->b", inputs, inputs, dtype=np.float64)
    ).astype(np.float32)
    xn = inputs / nrm[:, None]
    mv1, mv2a, mv2b = _gate_consts(thetas, phis, lams)

    nc = _get_nc()
    in_maps = [
        {
            "x": xn[k * B_CORE : (k + 1) * B_CORE],
            "mv1": mv1,
            "mv2a": mv2a,
            "mv2b": mv2b,
        }
        for k in range(N_CORES)
    ]
    res = run_bass_kernel_spmd(
        nc, in_maps, list(range(N_CORES)), trace=_trace, **(_trace_kwargs or {})
    )
    out = np.concatenate([res.results[k]["probs"] for k in range(N_CORES)], axis=0)
    if _trace:
        kernel.last_result = res
    return out
